# revision 19
# baseline (speedup 1.0000x reference)
"""Trainium2 Bass kernel for the wavelet-attention module (nn_ALW_55611236548963).

Strategy (pure data-parallel over batch, 16 samples per core x 8 cores):
  - Everything linear along the sequence axis is folded on the host into dense
    matrices: the 3-level db6 DWT becomes one [720, 751]-ish matrix D (applied
    as compT = D_aug^T @ x_aug with mean/std correction rows folded in), and
    the entire IDWT + per-scale output projection + positional-encoding bias
    collapses into per-scale G matrices [104, L_i] plus one OUTBIAS [321, 104].
  - Per-sample on-chip pipeline: stats (mean/var via ones-matmuls + Newton
    rsqrt on DVE), decomposition matmul (float32r, full rate at N=321),
    Q/K = tanh(comp @ qw^T + qb) with the per-channel 1/std folded into the
    activation's per-partition scale and qb/std folded in as an extra
    contraction row, scores + tanh, column means, then a batched-over-samples
    suffix-sum/softmax/soft-window stage, and finally one accumulated matmul
    producing the [321, 104] output directly.
"""

import numpy as np
import ml_dtypes

import concourse.bass as bass
import concourse.mybir as mybir
import concourse.tile as tile
from concourse.bass_utils import run_bass_kernel_spmd

# ---------------------------------------------------------------- drain fix
# walrus in this container rejects the TileContext tail Drain when it carries
# more than a couple of semaphore waits ("Too many sync wait commands").
# Spread the pending waits over sync NOPs (one wait each) before the drain.


def _patched_drain_and_barrier(self, tick_clock, wait_clock):
    import bass_rust

    nop = self.nc.sync.nop(nofuse=True, hint="drain_spread")
    wait_clock.add_sem_waits(nop.ins, bass_rust.ScopedClock({None: tick_clock.global_clock}))
    si = nop.ins.sync_info
    waits = list(si.on_wait) if si and si.on_wait else []
    if len(waits) > 1:
        si.on_wait = waits[:1]
        for w in waits[1:]:
            nxt = self.nc.sync.nop(nofuse=True, hint="drain_spread2")
            nsi = nxt.ins.sync_info
            if nsi is None:
                nxt.ins.sync_info = mybir.SyncInfo(on_wait=[w], on_update=[])
            else:
                nsi.on_wait = (list(nsi.on_wait) if nsi.on_wait else []) + [w]
    self.nc.sync.drain()
    self.nc.all_engine_barrier()
    popped = self.nc._tile_sem_poison_stack.pop()
    assert popped is self._sem_poison
    self.nc.clear_and_free_semaphores(list(self.sems.allocated().values()))
    self.nc.all_engine_barrier()


tile.TileContext._drain_and_barrier = _patched_drain_and_barrier

_MAX_WAITS = 1
SPLIT_WAITS = True  # CoreSim can't execute the injected NOPs; tests may disable
DEBUG_OUTPUTS = False
STOP_AFTER = ''  # 'A' or 'B' to truncate the kernel for bisection


def _split_excess_waits(nc):
    """walrus here accepts at most ~2 semaphore waits per instruction; move
    excess waits onto same-engine NOPs inserted just before the instruction."""
    wid = 0
    for fn in nc.m.functions:
        for bb in fn.blocks:
            new_insts = []
            for inst in bb.instructions:
                si = inst.sync_info
                waits = list(si.on_wait) if si and si.on_wait else []
                if len(waits) > _MAX_WAITS:
                    excess, keep = waits[:-_MAX_WAITS], waits[-_MAX_WAITS:]
                    for g0 in range(0, len(excess), _MAX_WAITS):
                        grp = excess[g0:g0 + _MAX_WAITS]
                        nop = mybir.InstNoOp(name=f"WSPL-{wid}", ins=[], outs=[])
                        wid += 1
                        nop.engine = inst.engine
                        nop.sync_info = mybir.SyncInfo(on_wait=grp, on_update=[])
                        new_insts.append(nop)
                    si.on_wait = keep
                new_insts.append(inst)
            bb.instructions[:] = new_insts

# ---------------------------------------------------------------- dimensions

N_CORES = 8
BATCH, SEQ, CH = 128, 720, 321
BPC = BATCH // N_CORES                     # 16 samples per core
LF = 12
COMP_L = [99, 365, 188, 99]                # lengths of [yl, yh1, yh2, yh3]
OUT_L = [22, 57, 34, 22]
OUTW = 104
NKB = 6                                    # x row blocks of 128 (last = 80)
CCH = [(0, 128), (128, 128), (256, 65)]    # channel chunks

_DB6 = np.array([0.11154074335008017, 0.4946238903983854, 0.7511339080215775,
                 0.3152503517092432, -0.22626469396516913, -0.12976686756709563,
                 0.09750160558707936, 0.02752286553001629, -0.031582039318031156,
                 0.0005538422009938016, 0.004777257511010651, -0.00107730108499558],
                dtype=np.float64)
_QMF = (_DB6[::-1].copy() * ((-1.0) ** np.arange(12)))

# compTu / D M-block layout: (scale, local_kb, row0, nrows, has_aug)
BLOCKS = [
    (1, 0, 0, 128, False),
    (1, 1, 128, 128, False),
    (1, 2, 256, 109, True),
    (2, 0, 0, 128, False),
    (2, 1, 128, 60, True),
    (3, 0, 0, 99, True),
    (0, 0, 0, 99, True),
]
SCALE_BLOCKS = {1: [0, 1, 2], 2: [3, 4], 3: [5], 0: [6]}
MW = [nr + (1 if aug else 0) for (_, _, _, nr, aug) in BLOCKS]
MOFF = np.concatenate([[0], np.cumsum(MW)]).astype(int)
MTOT = int(MOFF[-1])                       # 755

F32, BF16, F32R = mybir.dt.float32, mybir.dt.bfloat16, mybir.dt.float32r

# small-scale packing inside one PSUM bank: [s2 | s3 | s0] widths 188/99/99
SM_OFF = {2: 0, 3: 188, 0: 287}
SMW = 386


# ---------------------------------------------------------------- host math

def _afb_matrices(N):
    out = (N + LF - 1) // 2
    p = 2 * (out - 1) - N + LF
    pl = p // 2

    def src(i):
        j = (i - pl) % (2 * N)
        return j if j < N else 2 * N - 1 - j

    A_lo = np.zeros((N, out), np.float64)
    A_hi = np.zeros((N, out), np.float64)
    for k in range(out):
        for t in range(LF):
            r = src(2 * k + t)
            A_lo[r, k] += _DB6[t]
            A_hi[r, k] += _QMF[t]
    return A_lo, A_hi


def _sfb_matrices(Lin):
    Lout = 2 * Lin + 2 - LF
    S_lo = np.zeros((Lout, Lin), np.float64)
    S_hi = np.zeros((Lout, Lin), np.float64)
    for m in range(Lin):
        for t in range(LF):
            j = 2 * m + 1 - t
            if 0 <= j < Lout:
                S_lo[j, m] += _DB6[LF - 1 - t]
                S_hi[j, m] += _QMF[LF - 1 - t]
    return S_lo, S_hi


def _decomp_matrices():
    A1_lo, A1_hi = _afb_matrices(720)
    A2_lo, A2_hi = _afb_matrices(365)
    A3_lo, A3_hi = _afb_matrices(188)
    lo1 = A1_lo
    lo2 = lo1 @ A2_lo
    return [lo2 @ A3_lo, A1_hi, lo1 @ A2_hi, lo2 @ A3_hi]


def _recon_matrices():
    S22_lo, S22_hi = _sfb_matrices(22)
    S34_lo, S34_hi = _sfb_matrices(34)
    S57_lo, S57_hi = _sfb_matrices(57)
    M_p1 = S57_hi
    C34 = S57_lo @ S34_lo[:57, :]
    M_p2 = S57_lo @ S34_hi[:57, :]
    return [C34 @ S22_lo, M_p1, M_p2, C34 @ S22_hi]


def _sinusoidal_pe(n_pos, d):
    pos = np.arange(n_pos, dtype=np.float32)[:, None]
    div = np.exp(np.arange(0, d, 2, dtype=np.float32) * (-np.log(10000.0) / d))
    pe = np.zeros((n_pos, d), np.float32)
    pe[:, 0::2] = np.sin(pos * div)
    pe[:, 1::2] = np.cos(pos * div[: d // 2])
    return pe


def _to_blocks(mat, nblk, width):
    """[V, W] row-chunked to tile layout [128, nblk*W] (chunk kb at cols kb*W)."""
    V, W = mat.shape
    assert W == width and V <= nblk * 128
    arr = np.zeros((nblk, 128, W), mat.dtype)
    for kb in range(nblk):
        r0 = 128 * kb
        nr = min(128, V - r0)
        if nr > 0:
            arr[kb, :nr] = mat[r0:r0 + nr]
    return np.ascontiguousarray(arr.transpose(1, 0, 2).reshape(128, nblk * W))


def build_constants(inputs):
    """All host-folded constants, keyed by dram parameter name."""
    qws = [inputs[f"qw{i}"] for i in range(4)]
    qbs = [inputs[f"qb{i}"] for i in range(4)]
    kws = [inputs[f"kw{i}"] for i in range(4)]
    kbs = [inputs[f"kb{i}"] for i in range(4)]
    pws = [np.asarray(inputs[f"pw{i}"], np.float64) for i in range(4)]
    pbs = [np.asarray(inputs[f"pb{i}"], np.float64) for i in range(4)]
    iws = [np.asarray(inputs[f"iw{i}"], np.float32) for i in range(4)]

    for iw in iws:
        spread = np.ptp(iw, axis=1).max()
        if spread > 1e-6 * (np.abs(iw).max() + 1e-30):
            raise ValueError("iw varies along the channel axis; fast path invalid")

    Ds = _decomp_matrices()                 # [720, L_i]
    Rs = _recon_matrices()                  # [104, O_i]
    ci = 1.0 / CH                           # softmax of channel-constant iw

    # ---- D_aug lhsT tiles [6][128, MTOT] + skip map
    dl = np.zeros((NKB, 128, MTOT), np.float64)
    for mb, (si, kb_l, row0, nr, aug) in enumerate(BLOCKS):
        D = Ds[si]
        cols = D[:, row0:row0 + nr]          # [720, nr]
        s_col = cols.sum(axis=0)             # column sums for mean correction
        m0 = MOFF[mb]
        for k in range(NKB):
            r0 = 128 * k
            rr = min(128, SEQ - r0)
            dl[k, :rr, m0:m0 + nr] = cols[r0:r0 + rr]
        dl[5, 80, m0:m0 + nr] = -s_col       # mean-correction row
        if aug:
            dl[5, 81, m0 + nr] = 1.0         # std passthrough row
    include = {}
    for mb in range(len(BLOCKS)):
        m0, m1 = MOFF[mb], MOFF[mb + 1]
        include[mb] = [k for k in range(NKB) if np.any(dl[k, :, m0:m1] != 0.0)]

    consts = {
        "dlhs": np.ascontiguousarray(
            dl.transpose(1, 0, 2).reshape(128, NKB * MTOT)).astype(ml_dtypes.bfloat16),
    }

    # ---- qwT/kwT tiles with bias row folded at the aug position
    for i in range(4):
        L = COMP_L[i]
        lc = (L + 127) // 128
        for nm, w, b in (("q", qws[i], qbs[i]), ("k", kws[i], kbs[i])):
            m = np.zeros((lc * 128, L), np.float64)
            m[:L] = np.asarray(w, np.float64).T       # row l = w[:, l]
            m[L] = np.asarray(b, np.float64)          # bias row right after
            consts[f"{nm}wt{i}"] = _to_blocks(m, lc, L).astype(ml_dtypes.bfloat16)

    # ---- suffix-sum matrices (1/L folded in)
    for i in range(4):
        L = COMP_L[i]
        lc = (L + 127) // 128
        T = (np.arange(L)[:, None] >= np.arange(L)[None, :]).astype(np.float64) / L
        consts[f"tmat{i}"] = _to_blocks(T, lc, L).astype(ml_dtypes.bfloat16)

    # ---- G tiles [128, 7*104] (G_i = ci * R_i @ pw_i, transposed layout)
    GT = np.zeros((128, len(BLOCKS) * OUTW), np.float64)
    for mb, (si, kb_l, row0, nr, aug) in enumerate(BLOCKS):
        G = ci * (Rs[si] @ pws[si])          # [104, L_i]
        GT[:nr, mb * OUTW:(mb + 1) * OUTW] = G[:, row0:row0 + nr].T
    consts["gt"] = GT.astype(ml_dtypes.bfloat16)

    # ---- OUTBIAS [321, 104] -> [128, 3*104]
    ob = np.zeros((CH, OUTW), np.float64)
    for i in range(4):
        ob += ci * (np.ones((CH, 1)) @ (Rs[i] @ pbs[i])[None, :])
        ob += ci * (_sinusoidal_pe(CH, OUT_L[i]).astype(np.float64) @ Rs[i].T)
    obt = np.zeros((128, 3 * OUTW), np.float32)
    for mc, (c0, cw) in enumerate(CCH):
        obt[:cw, mc * OUTW:(mc + 1) * OUTW] = ob[c0:c0 + cw]
    consts["outbias"] = obt

    consts["idx16"] = np.broadcast_to(
        np.arange(365, dtype=np.float32), (BPC, 365)).copy()
    consts["ident16"] = np.eye(16, dtype=np.float32)
    oneh = np.zeros((128, 16 * BPC), ml_dtypes.bfloat16)
    for b in range(BPC):
        oneh[:, 16 * b + b] = 1.0
    consts["onehot16"] = oneh
    consts["onesbf"] = np.ones((128, 1), ml_dtypes.bfloat16)

    als = [float(np.asarray(inputs[f"al{i}"]).reshape(-1)[0]) for i in range(4)]
    bes = [float(np.asarray(inputs[f"be{i}"]).reshape(-1)[0]) for i in range(4)]
    return consts, include, als, bes


# ---------------------------------------------------------------- device code

CONST_SPECS = [
    ("dlhs", [128, NKB * MTOT], BF16),
    ("qwt1", [128, 3 * 365], BF16), ("qwt2", [128, 2 * 188], BF16),
    ("qwt3", [128, 99], BF16), ("qwt0", [128, 99], BF16),
    ("kwt1", [128, 3 * 365], BF16), ("kwt2", [128, 2 * 188], BF16),
    ("kwt3", [128, 99], BF16), ("kwt0", [128, 99], BF16),
    ("tmat1", [128, 3 * 365], BF16), ("tmat2", [128, 2 * 188], BF16),
    ("tmat3", [128, 99], BF16), ("tmat0", [128, 99], BF16),
    ("gt", [128, 7 * OUTW], BF16),
    ("outbias", [128, 3 * OUTW], F32),
    ("idx16", [BPC, 365], F32),
    ("ident16", [16, 16], F32),
    ("onehot16", [128, 16 * BPC], BF16),
    ("onesbf", [128, 1], BF16),
]


def build_nc(include, als, bes):
    nc = _build_nc_inner(include, als, bes)
    if SPLIT_WAITS:
        _split_excess_waits(nc)
    return nc


def _build_nc_inner(include, als, bes):
    nc = bass.Bass()
    x_p = nc.declare_dram_parameter("x", [BPC, SEQ, CH], BF16, isOutput=False)
    cparams = {}
    for nm, shp, dt in CONST_SPECS:
        cparams[nm] = nc.declare_dram_parameter(nm, shp, dt, isOutput=False)
    out_p = nc.declare_dram_parameter("out", [BPC, CH, OUTW], F32, isOutput=True)
    mean_p = nc.declare_dram_parameter("mean", [BPC, CH], F32, isOutput=True)
    std_p = nc.declare_dram_parameter("std", [BPC, CH], F32, isOutput=True)
    dbg = {}
    if DEBUG_OUTPUTS:
        for nm, shp in (("d_statrows", [BPC, 2 * CH]), ("d_rstd", [BPC, CH]),
                        ("d_comptu", [128, 7 * CH]), ("d_qs1", [128, 3 * 365]),
                        ("d_th1", [128, 3 * 365]), ("d_wt1", [128, 3 * BPC]),
                        ("d_cum1", [BPC, 365]), ("d_mask1", [BPC, 365]),
                        ("d_maskt", [128, 7 * BPC]), ("d_rstdt", [128, 3 * BPC])):
            dbg[nm] = nc.declare_dram_parameter(nm, shp, F32, isOutput=True)

    scales_order = [1, 2, 3, 0]
    LC = {i: (COMP_L[i] + 127) // 128 for i in range(4)}

    with tile.TileContext(nc) as tc:
        import contextlib
        with contextlib.ExitStack() as ctx:
            cpool = ctx.enter_context(tc.tile_pool(name="const", bufs=1))
            ppool = ctx.enter_context(tc.tile_pool(name="persist", bufs=1))

            # ---- constants into SBUF
            C = {}
            for nm, shp, dt in CONST_SPECS:
                t = cpool.tile(shp, dt, tag=nm, name=f"c_{nm}")
                nc.sync.dma_start(out=t[:], in_=cparams[nm][:])
                C[nm] = t

            # ---- persistent tiles
            compTu = [ppool.tile([128, 7 * CH], BF16, tag=f"compTu{b}",
                                 name=f"compTu{b}") for b in range(BPC)]
            WT16 = {i: ppool.tile([128, LC[i] * BPC], BF16, tag=f"wt16_{i}",
                                  name=f"wt16_{i}") for i in range(4)}
            statrows = ppool.tile([BPC, 2 * CH], F32, tag="statrows", name="statrows")
            meanstd = ppool.tile([BPC, 2 * CH], F32, tag="meanstd", name="meanstd")
            meanstdbf = ppool.tile([BPC, 2 * CH], BF16, tag="meanstdbf", name="meanstdbf")
            rstd16 = ppool.tile([BPC, CH], F32, tag="rstd16", name="rstd16")
            rstdT = ppool.tile([128, 3 * BPC], F32, tag="rstdT", name="rstdT")
            maskT = ppool.tile([128, 7 * BPC], F32, tag="maskT", name="maskT")
            mask16 = {i: ppool.tile([BPC, COMP_L[i]], F32, tag=f"m16_{i}",
                                    name=f"m16_{i}") for i in range(4)}

            xpool = ctx.enter_context(tc.tile_pool(name="xp", bufs=3))

            def load_x(b, with_meanstd):
                xt = xpool.tile([128, NKB * CH], BF16, tag="xt", name="xt")
                src = x_p[b, 0:640].rearrange("(k p) c -> p k c", p=128)
                dst = xt[0:128, 0:5 * CH].rearrange("p (k c) -> p k c", c=CH)
                nc.sync.dma_start(out=dst, in_=src)
                nc.sync.dma_start(out=xt[0:80, 5 * CH:6 * CH], in_=x_p[b, 640:720])
                if with_meanstd:
                    nc.sync.dma_start(out=xt[80:81, 5 * CH:6 * CH],
                                      in_=meanstdbf[b:b + 1, 0:CH])
                    nc.sync.dma_start(out=xt[81:82, 5 * CH:6 * CH],
                                      in_=meanstdbf[b:b + 1, CH:2 * CH])
                return xt

            # ================= Phase A: stats =================
            # Sums accumulate into PSUM partition b via a one-hot stationary
            # operand, so each sample's stats land in its own partition row.
            with tc.tile_pool(name="pa", bufs=2) as papool, \
                 tc.tile_pool(name="pap", bufs=1, space="PSUM") as paps, \
                 tc.tile_pool(name="papt", bufs=2, space="PSUM") as papt:
                ps_s = paps.tile([BPC, CH], F32, tag="pss", name="pss")
                ps_q = paps.tile([BPC, CH], F32, tag="psq", name="psq")
                for b in range(BPC):
                    xt = load_x(b, False)
                    xsq = papool.tile([128, NKB * CH], BF16, tag="xsq", name="xsq")
                    nc.vector.tensor_mul(xsq[:, 0:5 * CH], xt[:, 0:5 * CH],
                                         xt[:, 0:5 * CH])
                    nc.vector.tensor_mul(xsq[0:80, 5 * CH:6 * CH],
                                         xt[0:80, 5 * CH:6 * CH],
                                         xt[0:80, 5 * CH:6 * CH])
                    for k in range(NKB):
                        kp = 128 if k < 5 else 80
                        first = (b == 0 and k == 0)
                        last = (b == BPC - 1 and k == 5)
                        oh = C["onehot16"][0:kp, 16 * b:16 * b + 16]
                        nc.tensor.matmul(
                            ps_s[0:BPC, :], oh,
                            xt[0:kp, k * CH:(k + 1) * CH],
                            start=first, stop=last)
                        nc.tensor.matmul(
                            ps_q[0:BPC, :], oh,
                            xsq[0:kp, k * CH:(k + 1) * CH],
                            start=first, stop=last)
                nc.vector.tensor_copy(statrows[:, 0:CH], ps_s[0:BPC, :])
                nc.vector.tensor_copy(statrows[:, CH:2 * CH], ps_q[0:BPC, :])

                # batched: mean, var, rstd (Newton), std
                t1 = papool.tile([BPC, CH], F32, tag="t1", name="t1")
                t2 = papool.tile([BPC, CH], F32, tag="t2", name="t2")
                t3 = papool.tile([BPC, CH], F32, tag="t3", name="t3")
                nc.vector.tensor_scalar_mul(meanstd[:, 0:CH], statrows[:, 0:CH],
                                            1.0 / SEQ)
                nc.vector.tensor_scalar_mul(t1[:], statrows[:, CH:2 * CH], 1.0 / SEQ)
                nc.vector.tensor_mul(t2[:], meanstd[:, 0:CH], meanstd[:, 0:CH])
                # v = (msq + eps) - mean^2
                nc.vector.scalar_tensor_tensor(
                    t1[:], t1[:], 1e-5, t2[:],
                    op0=mybir.AluOpType.add, op1=mybir.AluOpType.subtract)
                # Newton rsqrt: y1 = 1.5 - 0.5 v ; 4 refinement steps
                y = rstd16
                nc.vector.tensor_scalar(y[:], t1[:], -0.5, 1.5,
                                        op0=mybir.AluOpType.mult,
                                        op1=mybir.AluOpType.add)
                for it in range(4):
                    nc.vector.tensor_mul(t2[:], y[:], y[:])
                    nc.vector.tensor_mul(t3[:], t2[:], t1[:])
                    nc.vector.tensor_scalar(t3[:], t3[:], -0.5, 1.5,
                                            op0=mybir.AluOpType.mult,
                                            op1=mybir.AluOpType.add)
                    nc.vector.tensor_mul(y[:], y[:], t3[:])
                nc.vector.tensor_mul(meanstd[:, CH:2 * CH], t1[:], y[:])  # std = v*y
                nc.vector.tensor_copy(meanstdbf[:], meanstd[:])
                if DEBUG_OUTPUTS:
                    nc.sync.dma_start(out=dbg["d_statrows"][:], in_=statrows[:])
                    nc.sync.dma_start(out=dbg["d_rstd"][:], in_=rstd16[:])
                nc.sync.dma_start(out=mean_p[:], in_=meanstd[:, 0:CH])
                nc.sync.dma_start(out=std_p[:], in_=meanstd[:, CH:2 * CH])
                # rstdT: [16, 321] -> [321, 16] in c-chunks
                for mc, (c0, cw) in enumerate(CCH):
                    pst = papt.tile([128, 16], F32, tag="ptr", name="ptr")
                    nc.tensor.transpose(pst[0:cw, 0:16],
                                        rstd16[0:BPC, c0:c0 + cw],
                                        C["ident16"][:])
                    nc.scalar.copy(rstdT[0:cw, mc * BPC:(mc + 1) * BPC],
                                   pst[0:cw, 0:16])
                if DEBUG_OUTPUTS:
                    nc.sync.dma_start(out=dbg["d_rstdt"][:], in_=rstdT[:])

            # ================= Phase B: per-sample heavy pipeline ============
            if STOP_AFTER == 'A':
                return nc
            with tc.tile_pool(name="pb", bufs=2) as pbpool, \
                 tc.tile_pool(name="pbp", bufs=2, space="PSUM") as pd, \
                 tc.tile_pool(name="pqkp", bufs=2, space="PSUM") as pqk, \
                 tc.tile_pool(name="psp", bufs=2, space="PSUM") as psc, \
                 tc.tile_pool(name="pwp", bufs=2, space="PSUM") as pwt:
                for b in range(BPC):
                    xt = load_x(b, True)
                    cu = compTu[b]
                    # ---- decomposition matmuls (float32r) + copies
                    for mb, (si, kb_l, row0, nr, aug) in enumerate(BLOCKS):
                        mw = MW[mb]
                        m0 = MOFF[mb]
                        ps = pd.tile([128, CH], F32, tag="pd", name="pd")
                        incl = include[mb]
                        for j, k in enumerate(incl):
                            kp = 128 if k < 5 else 82
                            nc.tensor.matmul(
                                ps[0:mw, :],
                                C["dlhs"][0:kp, k * MTOT + m0:k * MTOT + m0 + mw],
                                xt[0:kp, k * CH:(k + 1) * CH],
                                start=(j == 0), stop=(j == len(incl) - 1))
                        nc.vector.tensor_copy(cu[0:mw, mb * CH:(mb + 1) * CH],
                                              ps[0:mw, :])

                    # ---- Q/K with tanh (rstd as per-partition scale)
                    qk_sb = {}
                    for nm in ("q", "k"):
                        s1 = pbpool.tile([128, 3 * 365], BF16, tag=f"{nm}s1",
                                         name=f"{nm}s1")
                        sm = pbpool.tile([128, 3 * SMW], BF16, tag=f"{nm}sm",
                                         name=f"{nm}sm")
                        qk_sb[nm] = (s1, sm)
                        for mc, (c0, cw) in enumerate(CCH):
                            scale_ap = rstdT[0:cw, mc * BPC + b:mc * BPC + b + 1]
                            # scale 1 alone
                            pq = pqk.tile([128, SMW], F32, tag="pqk", name="pqk")
                            for jj, mb in enumerate(SCALE_BLOCKS[1]):
                                _, kb_l, row0, nr, aug = BLOCKS[mb]
                                kr = nr + (1 if aug else 0)
                                nc.tensor.matmul(
                                    pq[0:cw, 0:365],
                                    cu[0:kr, mb * CH + c0:mb * CH + c0 + cw],
                                    C[f"{nm}wt1"][0:kr, kb_l * 365:(kb_l + 1) * 365],
                                    start=(jj == 0), stop=(jj == 2))
                            nc.scalar.activation(
                                s1[0:cw, mc * 365:(mc + 1) * 365], pq[0:cw, 0:365],
                                mybir.ActivationFunctionType.Tanh, scale=scale_ap)
                            # small scales share a bank
                            pq2 = pqk.tile([128, SMW], F32, tag="pqk", name="pqk2")
                            for si in (2, 3, 0):
                                o = SM_OFF[si]
                                L = COMP_L[si]
                                mbs = SCALE_BLOCKS[si]
                                for jj, mb in enumerate(mbs):
                                    _, kb_l, row0, nr, aug = BLOCKS[mb]
                                    kr = nr + (1 if aug else 0)
                                    nc.tensor.matmul(
                                        pq2[0:cw, o:o + L],
                                        cu[0:kr, mb * CH + c0:mb * CH + c0 + cw],
                                        C[f"{nm}wt{si}"][0:kr, kb_l * L:(kb_l + 1) * L],
                                        start=(jj == 0), stop=(jj == len(mbs) - 1))
                            nc.scalar.activation(
                                sm[0:cw, mc * SMW:(mc + 1) * SMW], pq2[0:cw, 0:SMW],
                                mybir.ActivationFunctionType.Tanh, scale=scale_ap)

                    (qs1, qsm), (ks1, ksm) = qk_sb["q"], qk_sb["k"]

                    # ---- scores + tanh -> tanhS (bf16)
                    th1 = pbpool.tile([128, 3 * 365], BF16, tag="th1", name="th1")
                    thm = pbpool.tile([128, SMW + 188], BF16, tag="thm", name="thm")
                    for ml in range(3):               # scale 1, l-chunks
                        lw = [128, 128, 109][ml]
                        pss = psc.tile([128, SMW], F32, tag="ps", name="pss")
                        for mc, (c0, cw) in enumerate(CCH):
                            nc.tensor.matmul(
                                pss[0:lw, 0:365],
                                qs1[0:cw, mc * 365 + 128 * ml:
                                    mc * 365 + 128 * ml + lw],
                                ks1[0:cw, mc * 365:(mc + 1) * 365],
                                start=(mc == 0), stop=(mc == 2))
                        nc.scalar.activation(
                            th1[0:lw, ml * 365:ml * 365 + 365], pss[0:lw, 0:365],
                            mybir.ActivationFunctionType.Tanh,
                            scale=float(1.0 / np.sqrt(365.0)))
                    psa = psc.tile([128, SMW], F32, tag="ps", name="psa")
                    psb = psc.tile([128, SMW], F32, tag="ps", name="psb")
                    for si, lw, qoff, dsto, dstp in (
                            (2, 128, SM_OFF[2], 0, psa),
                            (3, 99, SM_OFF[3], 188, psa),
                            (0, 99, SM_OFF[0], 287, psa),
                            (2, 60, SM_OFF[2] + 128, 0, psb)):
                        L = COMP_L[si]
                        for mc, (c0, cw) in enumerate(CCH):
                            nc.tensor.matmul(
                                dstp[0:lw, dsto:dsto + L],
                                qsm[0:cw, mc * SMW + qoff:mc * SMW + qoff + lw],
                                ksm[0:cw, mc * SMW + SM_OFF[si]:
                                    mc * SMW + SM_OFF[si] + L],
                                start=(mc == 0), stop=(mc == 2))
                    nc.scalar.activation(
                        thm[0:128, 0:188], psa[0:128, 0:188],
                        mybir.ActivationFunctionType.Tanh,
                        scale=float(1.0 / np.sqrt(188.0)))
                    nc.scalar.activation(
                        thm[0:99, 188:386], psa[0:99, 188:386],
                        mybir.ActivationFunctionType.Tanh,
                        scale=float(1.0 / np.sqrt(99.0)))
                    nc.scalar.activation(
                        thm[0:60, SMW:SMW + 188], psb[0:60, 0:188],
                        mybir.ActivationFunctionType.Tanh,
                        scale=float(1.0 / np.sqrt(188.0)))

                    # ---- column means over l -> WT16 columns
                    def wt_mms(si, lhs_list):
                        """lhs_list: per l-chunk (ap, lw) covering [lw, L]"""
                        L = COMP_L[si]
                        nchunk = (L + 127) // 128
                        for ms in range(nchunk):
                            sw = min(128, L - 128 * ms)
                            pw = pwt.tile([128, 1], F32, tag="pw", name="pw")
                            for jj, (ap, lw) in enumerate(lhs_list):
                                nc.tensor.matmul(
                                    pw[0:sw, 0:1],
                                    ap[0:lw, 128 * ms:128 * ms + sw],
                                    C["onesbf"][0:lw, 0:1],
                                    start=(jj == 0), stop=(jj == len(lhs_list) - 1))
                            nc.vector.tensor_copy(
                                WT16[si][0:sw, ms * BPC + b:ms * BPC + b + 1],
                                pw[0:sw, 0:1])

                    wt_mms(1, [(th1[:, 0:365], 128), (th1[:, 365:730], 128),
                               (th1[:, 730:1095], 109)])
                    wt_mms(2, [(thm[:, 0:188], 128), (thm[:, SMW:SMW + 188], 60)])
                    wt_mms(3, [(thm[:, 188:287], 99)])
                    wt_mms(0, [(thm[:, 287:386], 99)])
                    if DEBUG_OUTPUTS and b == 0:
                        nc.gpsimd.dma_start(out=dbg["d_comptu"][:], in_=cu[:])
                        nc.gpsimd.dma_start(out=dbg["d_qs1"][:], in_=qs1[:])
                        nc.gpsimd.dma_start(out=dbg["d_th1"][:], in_=th1[:])

            # ================= Phase C: batched softmax / window =============
            if STOP_AFTER == 'B':
                return nc
            with tc.tile_pool(name="pc", bufs=2) as pcpool, \
                 tc.tile_pool(name="pcp", bufs=2, space="PSUM") as pcps:
                for si in scales_order:
                    L = COMP_L[si]
                    lc = LC[si]
                    pscm = pcps.tile([BPC, 512], F32, tag="pcum", name="pcum")
                    for kb in range(lc):
                        tw = min(128, L - 128 * kb)
                        lhsT = WT16[si][0:tw, kb * BPC:(kb + 1) * BPC]
                        rhs = C[f"tmat{si}"][0:tw, kb * L:(kb + 1) * L]
                        nc.tensor.matmul(pscm[0:BPC, 0:L], lhsT, rhs,
                                         start=(kb == 0), stop=(kb == lc - 1))
                    rmax = pcpool.tile([BPC, 1], F32, tag="rmax", name="rmax")
                    nc.vector.tensor_reduce(rmax[:], pscm[0:BPC, 0:L],
                                            axis=mybir.AxisListType.X,
                                            op=mybir.AluOpType.max)
                    nbias = pcpool.tile([BPC, 1], F32, tag="nbias", name="nbias")
                    nc.vector.tensor_scalar_mul(nbias[:], rmax[:], -als[si])
                    e16 = pcpool.tile([BPC, 365], F32, tag="e16", name="e16")
                    nc.scalar.activation(e16[0:BPC, 0:L], pscm[0:BPC, 0:L],
                                         mybir.ActivationFunctionType.Exp,
                                         bias=nbias[:], scale=als[si])
                    s16 = pcpool.tile([BPC, 1], F32, tag="s16", name="s16")
                    nc.vector.tensor_reduce(s16[:], e16[0:BPC, 0:L],
                                            axis=mybir.AxisListType.X,
                                            op=mybir.AluOpType.add)
                    prod = pcpool.tile([BPC, 365], F32, tag="prod", name="prod")
                    n16 = pcpool.tile([BPC, 1], F32, tag="n16", name="n16")
                    nc.vector.tensor_mul(prod[0:BPC, 0:L], e16[0:BPC, 0:L],
                                         C["idx16"][0:BPC, 0:L])
                    nc.vector.tensor_reduce(n16[:], prod[0:BPC, 0:L],
                                            axis=mybir.AxisListType.X,
                                            op=mybir.AluOpType.add)
                    rs = pcpool.tile([BPC, 1], F32, tag="rs", name="rs")
                    nc.vector.reciprocal(rs[:], s16[:])
                    win = pcpool.tile([BPC, 1], F32, tag="win", name="win")
                    nc.vector.tensor_mul(win[:], n16[:], rs[:])
                    tb = pcpool.tile([BPC, 1], F32, tag="tb", name="tb")
                    nc.vector.tensor_scalar_mul(tb[:], win[:], -bes[si] / 2.0)
                    nc.scalar.activation(mask16[si][0:BPC, 0:L],
                                         C["idx16"][0:BPC, 0:L],
                                         mybir.ActivationFunctionType.Tanh,
                                         bias=tb[:], scale=bes[si] / 2.0)
                    if DEBUG_OUTPUTS and si == 1:
                        nc.gpsimd.dma_start(out=dbg["d_wt1"][:], in_=WT16[1][:])
                        nc.vector.tensor_copy(prod[0:BPC, 0:L], pscm[0:BPC, 0:L])
                        nc.sync.dma_start(out=dbg["d_cum1"][0:BPC, 0:L], in_=prod[0:BPC, 0:L])
                        nc.sync.dma_start(out=dbg["d_mask1"][0:BPC, 0:L], in_=mask16[1][0:BPC, 0:L])
                # transposes to maskT (+ affine 0.5 x + 0.5)
                for mb, (si, kb_l, row0, nr, aug) in enumerate(BLOCKS):
                    pst = pcps.tile([128, 16], F32, tag="ptm", name="ptm")
                    nc.tensor.transpose(pst[0:nr, 0:16],
                                        mask16[si][0:BPC, 128 * kb_l:128 * kb_l + nr],
                                        C["ident16"][:])
                    nc.scalar.activation(maskT[0:nr, mb * BPC:(mb + 1) * BPC],
                                         pst[0:nr, 0:16],
                                         mybir.ActivationFunctionType.Copy,
                                         bias=0.5, scale=0.5)
                if DEBUG_OUTPUTS:
                    nc.sync.dma_start(out=dbg["d_maskt"][:], in_=maskT[:])

            # ================= Phase D: masked projection to output ==========
            with tc.tile_pool(name="pdl", bufs=3) as pdpool, \
                 tc.tile_pool(name="pdp", bufs=2, space="PSUM") as pdps:
                for b in range(BPC):
                    cu = compTu[b]
                    gm = pdpool.tile([128, 7 * OUTW], BF16, tag="gm", name="gm")
                    for mb, (si, kb_l, row0, nr, aug) in enumerate(BLOCKS):
                        nc.vector.tensor_scalar_mul(
                            gm[0:nr, mb * OUTW:(mb + 1) * OUTW],
                            C["gt"][0:nr, mb * OUTW:(mb + 1) * OUTW],
                            maskT[0:nr, mb * BPC + b:mb * BPC + b + 1])
                    po = pdps.tile([128, 3 * OUTW], F32, tag="po", name="po")
                    for mc, (c0, cw) in enumerate(CCH):
                        for mb in range(7):
                            nr = BLOCKS[mb][3]
                            nc.tensor.matmul(
                                po[0:cw, mc * OUTW:(mc + 1) * OUTW],
                                cu[0:nr, mb * CH + c0:mb * CH + c0 + cw],
                                gm[0:nr, mb * OUTW:(mb + 1) * OUTW],
                                start=(mb == 0), stop=(mb == 6))
                    osb = pdpool.tile([128, 3 * OUTW], F32, tag="osb", name="osb")
                    for mc, (c0, cw) in enumerate(CCH):
                        nc.vector.scalar_tensor_tensor(
                            osb[0:cw, mc * OUTW:(mc + 1) * OUTW],
                            po[0:cw, mc * OUTW:(mc + 1) * OUTW],
                            rstdT[0:cw, mc * BPC + b:mc * BPC + b + 1],
                            C["outbias"][0:cw, mc * OUTW:(mc + 1) * OUTW],
                            op0=mybir.AluOpType.mult, op1=mybir.AluOpType.add)
                    dst = out_p[b, 0:256].rearrange("(k p) j -> p k j", p=128)
                    src = osb[0:128, 0:2 * OUTW].rearrange("p (k j) -> p k j", j=OUTW)
                    nc.sync.dma_start(out=dst, in_=src)
                    nc.sync.dma_start(out=out_p[b, 256:CH],
                                      in_=osb[0:65, 2 * OUTW:3 * OUTW])
    return nc


# ---------------------------------------------------------------- entry point

def kernel(**inputs):
    x = np.ascontiguousarray(np.asarray(inputs["x"], np.float32))
    consts, include, als, bes = build_constants(inputs)
    nc = build_nc(include, als, bes)
    in_maps = []
    for c in range(N_CORES):
        m = dict(consts)
        m["x"] = np.ascontiguousarray(x[c * BPC:(c + 1) * BPC]).astype(
            ml_dtypes.bfloat16)
        in_maps.append(m)
    res = run_bass_kernel_spmd(nc, in_maps, list(range(N_CORES)))
    out = np.concatenate([res.results[c]["out"] for c in range(N_CORES)], axis=0)
    mean = np.concatenate([res.results[c]["mean"] for c in range(N_CORES)], axis=0)
    std = np.concatenate([res.results[c]["std"] for c in range(N_CORES)], axis=0)
    return (out.astype(np.float32),
            mean.reshape(BATCH, 1, CH).astype(np.float32),
            std.reshape(BATCH, 1, CH).astype(np.float32))


# revision 23
# speedup vs baseline: 3.0660x; 3.0660x over previous
"""Trainium2 Bass kernel for the wavelet-attention module (nn_ALW_55611236548963).

Strategy (pure data-parallel over batch, 16 samples per core x 8 cores):
  - Everything linear along the sequence axis is folded on the host into dense
    matrices: the 3-level db6 DWT becomes one [720, 751]-ish matrix D (applied
    as compT = D_aug^T @ x_aug with mean/std correction rows folded in), and
    the entire IDWT + per-scale output projection + positional-encoding bias
    collapses into per-scale G matrices [104, L_i] plus one OUTBIAS [321, 104].
  - Per-sample on-chip pipeline: stats (mean/var via ones-matmuls + Newton
    rsqrt on DVE), decomposition matmul (bf16),
    Q/K = tanh(comp @ qw^T + qb) with the per-channel 1/std folded into the
    activation's per-partition scale and qb/std folded in as an extra
    contraction row, scores + tanh, column means, then a batched-over-samples
    suffix-sum/softmax/soft-window stage, and finally one accumulated matmul
    producing the [321, 104] output directly.
"""

import numpy as np
import ml_dtypes

import concourse.bass as bass
import concourse.mybir as mybir
import concourse.tile as tile
from concourse.bass_utils import run_bass_kernel_spmd

# ---------------------------------------------------------------- drain fix
# walrus in this container rejects the TileContext tail Drain when it carries
# more than a couple of semaphore waits ("Too many sync wait commands").
# Spread the pending waits over sync NOPs (one wait each) before the drain.


def _patched_drain_and_barrier(self, tick_clock, wait_clock):
    import bass_rust

    nop = self.nc.sync.nop(nofuse=True, hint="drain_spread")
    wait_clock.add_sem_waits(nop.ins, bass_rust.ScopedClock({None: tick_clock.global_clock}))
    si = nop.ins.sync_info
    waits = list(si.on_wait) if si and si.on_wait else []
    if len(waits) > 1:
        si.on_wait = waits[:1]
        for w in waits[1:]:
            nxt = self.nc.sync.nop(nofuse=True, hint="drain_spread2")
            nsi = nxt.ins.sync_info
            if nsi is None:
                nxt.ins.sync_info = mybir.SyncInfo(on_wait=[w], on_update=[])
            else:
                nsi.on_wait = (list(nsi.on_wait) if nsi.on_wait else []) + [w]
    self.nc.sync.drain()
    self.nc.all_engine_barrier()
    popped = self.nc._tile_sem_poison_stack.pop()
    assert popped is self._sem_poison
    self.nc.clear_and_free_semaphores(list(self.sems.allocated().values()))
    self.nc.all_engine_barrier()


tile.TileContext._drain_and_barrier = _patched_drain_and_barrier

_MAX_WAITS = 1
SPLIT_WAITS = True  # CoreSim can't execute the injected NOPs; tests may disable
DEBUG_OUTPUTS = False
STOP_AFTER = ''  # 'A' or 'B' to truncate the kernel for bisection


def _split_excess_waits(nc):
    """walrus here accepts at most ~2 semaphore waits per instruction; move
    excess waits onto same-engine NOPs inserted just before the instruction."""
    wid = 0
    for fn in nc.m.functions:
        for bb in fn.blocks:
            new_insts = []
            for inst in bb.instructions:
                si = inst.sync_info
                waits = list(si.on_wait) if si and si.on_wait else []
                if len(waits) > _MAX_WAITS:
                    excess, keep = waits[:-_MAX_WAITS], waits[-_MAX_WAITS:]
                    for g0 in range(0, len(excess), _MAX_WAITS):
                        grp = excess[g0:g0 + _MAX_WAITS]
                        nop = mybir.InstNoOp(name=f"WSPL-{wid}", ins=[], outs=[])
                        wid += 1
                        nop.engine = inst.engine
                        nop.sync_info = mybir.SyncInfo(on_wait=grp, on_update=[])
                        new_insts.append(nop)
                    si.on_wait = keep
                new_insts.append(inst)
            bb.instructions[:] = new_insts

# ---------------------------------------------------------------- dimensions

N_CORES = 8
BATCH, SEQ, CH = 128, 720, 321
BPC = BATCH // N_CORES                     # 16 samples per core
LF = 12
COMP_L = [99, 365, 188, 99]                # lengths of [yl, yh1, yh2, yh3]
OUT_L = [22, 57, 34, 22]
OUTW = 104
NKB = 6                                    # x row blocks of 128 (last = 80)
CCH = [(0, 128), (128, 128), (256, 65)]    # channel chunks

_DB6 = np.array([0.11154074335008017, 0.4946238903983854, 0.7511339080215775,
                 0.3152503517092432, -0.22626469396516913, -0.12976686756709563,
                 0.09750160558707936, 0.02752286553001629, -0.031582039318031156,
                 0.0005538422009938016, 0.004777257511010651, -0.00107730108499558],
                dtype=np.float64)
_QMF = (_DB6[::-1].copy() * ((-1.0) ** np.arange(12)))

# compTu / D M-block layout: (scale, local_kb, row0, nrows, has_aug)
BLOCKS = [
    (1, 0, 0, 128, False),
    (1, 1, 128, 128, False),
    (1, 2, 256, 109, True),
    (2, 0, 0, 128, False),
    (2, 1, 128, 60, True),
    (3, 0, 0, 99, True),
    (0, 0, 0, 99, True),
]
SCALE_BLOCKS = {1: [0, 1, 2], 2: [3, 4], 3: [5], 0: [6]}
MW = [nr + (1 if aug else 0) for (_, _, _, nr, aug) in BLOCKS]
MOFF = np.concatenate([[0], np.cumsum(MW)]).astype(int)
MTOT = int(MOFF[-1])                       # 755

F32, BF16, F32R = mybir.dt.float32, mybir.dt.bfloat16, mybir.dt.float32r

# small-scale packing inside one PSUM bank: [s2 | s3 | s0] widths 188/99/99
SM_OFF = {2: 0, 3: 188, 0: 287}
SMW = 386


# ---------------------------------------------------------------- host math

def _afb_matrices(N):
    out = (N + LF - 1) // 2
    p = 2 * (out - 1) - N + LF
    pl = p // 2

    def src(i):
        j = (i - pl) % (2 * N)
        return j if j < N else 2 * N - 1 - j

    A_lo = np.zeros((N, out), np.float64)
    A_hi = np.zeros((N, out), np.float64)
    for k in range(out):
        for t in range(LF):
            r = src(2 * k + t)
            A_lo[r, k] += _DB6[t]
            A_hi[r, k] += _QMF[t]
    return A_lo, A_hi


def _sfb_matrices(Lin):
    Lout = 2 * Lin + 2 - LF
    S_lo = np.zeros((Lout, Lin), np.float64)
    S_hi = np.zeros((Lout, Lin), np.float64)
    for m in range(Lin):
        for t in range(LF):
            j = 2 * m + 1 - t
            if 0 <= j < Lout:
                S_lo[j, m] += _DB6[LF - 1 - t]
                S_hi[j, m] += _QMF[LF - 1 - t]
    return S_lo, S_hi


def _decomp_matrices():
    A1_lo, A1_hi = _afb_matrices(720)
    A2_lo, A2_hi = _afb_matrices(365)
    A3_lo, A3_hi = _afb_matrices(188)
    lo1 = A1_lo
    lo2 = lo1 @ A2_lo
    return [lo2 @ A3_lo, A1_hi, lo1 @ A2_hi, lo2 @ A3_hi]


def _recon_matrices():
    S22_lo, S22_hi = _sfb_matrices(22)
    S34_lo, S34_hi = _sfb_matrices(34)
    S57_lo, S57_hi = _sfb_matrices(57)
    M_p1 = S57_hi
    C34 = S57_lo @ S34_lo[:57, :]
    M_p2 = S57_lo @ S34_hi[:57, :]
    return [C34 @ S22_lo, M_p1, M_p2, C34 @ S22_hi]


def _sinusoidal_pe(n_pos, d):
    pos = np.arange(n_pos, dtype=np.float32)[:, None]
    div = np.exp(np.arange(0, d, 2, dtype=np.float32) * (-np.log(10000.0) / d))
    pe = np.zeros((n_pos, d), np.float32)
    pe[:, 0::2] = np.sin(pos * div)
    pe[:, 1::2] = np.cos(pos * div[: d // 2])
    return pe


def _to_blocks(mat, nblk, width):
    """[V, W] row-chunked to tile layout [128, nblk*W] (chunk kb at cols kb*W)."""
    V, W = mat.shape
    assert W == width and V <= nblk * 128
    arr = np.zeros((nblk, 128, W), mat.dtype)
    for kb in range(nblk):
        r0 = 128 * kb
        nr = min(128, V - r0)
        if nr > 0:
            arr[kb, :nr] = mat[r0:r0 + nr]
    return np.ascontiguousarray(arr.transpose(1, 0, 2).reshape(128, nblk * W))


def build_constants(inputs):
    """All host-folded constants, keyed by dram parameter name."""
    qws = [inputs[f"qw{i}"] for i in range(4)]
    qbs = [inputs[f"qb{i}"] for i in range(4)]
    kws = [inputs[f"kw{i}"] for i in range(4)]
    kbs = [inputs[f"kb{i}"] for i in range(4)]
    pws = [np.asarray(inputs[f"pw{i}"], np.float64) for i in range(4)]
    pbs = [np.asarray(inputs[f"pb{i}"], np.float64) for i in range(4)]
    iws = [np.asarray(inputs[f"iw{i}"], np.float32) for i in range(4)]

    for iw in iws:
        spread = np.ptp(iw, axis=1).max()
        if spread > 1e-6 * (np.abs(iw).max() + 1e-30):
            raise ValueError("iw varies along the channel axis; fast path invalid")

    Ds = _decomp_matrices()                 # [720, L_i]
    Rs = _recon_matrices()                  # [104, O_i]
    ci = 1.0 / CH                           # softmax of channel-constant iw

    # ---- D_aug lhsT tiles [6][128, MTOT] + skip map
    dl = np.zeros((NKB, 128, MTOT), np.float64)
    for mb, (si, kb_l, row0, nr, aug) in enumerate(BLOCKS):
        D = Ds[si]
        cols = D[:, row0:row0 + nr]          # [720, nr]
        s_col = cols.sum(axis=0)             # column sums for mean correction
        m0 = MOFF[mb]
        for k in range(NKB):
            r0 = 128 * k
            rr = min(128, SEQ - r0)
            dl[k, :rr, m0:m0 + nr] = cols[r0:r0 + rr]
        dl[5, 80, m0:m0 + nr] = -s_col       # mean-correction row
        if aug:
            dl[5, 81, m0 + nr] = 1.0         # std passthrough row
    include = {}
    for mb in range(len(BLOCKS)):
        m0, m1 = MOFF[mb], MOFF[mb + 1]
        include[mb] = [k for k in range(NKB) if np.any(dl[k, :, m0:m1] != 0.0)]

    consts = {
        "dlhs": np.ascontiguousarray(
            dl.transpose(1, 0, 2).reshape(128, NKB * MTOT)).astype(ml_dtypes.bfloat16),
    }

    # ---- qwT/kwT tiles with bias row folded at the aug position
    for i in range(4):
        L = COMP_L[i]
        lc = (L + 127) // 128
        for nm, w, b in (("q", qws[i], qbs[i]), ("k", kws[i], kbs[i])):
            m = np.zeros((lc * 128, L), np.float64)
            m[:L] = np.asarray(w, np.float64).T       # row l = w[:, l]
            m[L] = np.asarray(b, np.float64)          # bias row right after
            consts[f"{nm}wt{i}"] = _to_blocks(m, lc, L).astype(ml_dtypes.bfloat16)

    # ---- suffix-sum matrices (1/L folded in)
    for i in range(4):
        L = COMP_L[i]
        lc = (L + 127) // 128
        T = (np.arange(L)[:, None] >= np.arange(L)[None, :]).astype(np.float64) / L
        consts[f"tmat{i}"] = _to_blocks(T, lc, L).astype(ml_dtypes.bfloat16)

    # ---- G tiles [128, 7*104] (G_i = ci * R_i @ pw_i, transposed layout)
    GT = np.zeros((128, len(BLOCKS) * OUTW), np.float64)
    for mb, (si, kb_l, row0, nr, aug) in enumerate(BLOCKS):
        G = ci * (Rs[si] @ pws[si])          # [104, L_i]
        GT[:nr, mb * OUTW:(mb + 1) * OUTW] = G[:, row0:row0 + nr].T
    consts["gt"] = GT.astype(ml_dtypes.bfloat16)

    # ---- OUTBIAS [321, 104] -> [128, 3*104]
    ob = np.zeros((CH, OUTW), np.float64)
    for i in range(4):
        ob += ci * (np.ones((CH, 1)) @ (Rs[i] @ pbs[i])[None, :])
        ob += ci * (_sinusoidal_pe(CH, OUT_L[i]).astype(np.float64) @ Rs[i].T)
    obt = np.zeros((128, 3 * OUTW), np.float32)
    for mc, (c0, cw) in enumerate(CCH):
        obt[:cw, mc * OUTW:(mc + 1) * OUTW] = ob[c0:c0 + cw]
    consts["outbias"] = obt

    consts["idx16"] = np.broadcast_to(
        np.arange(365, dtype=np.float32), (BPC, 365)).copy()
    consts["ident16"] = np.eye(16, dtype=np.float32)
    oneh = np.zeros((128, 16 * BPC), ml_dtypes.bfloat16)
    for b in range(BPC):
        oneh[:, 16 * b + b] = 1.0
    consts["onehot16"] = oneh
    consts["onesbf"] = np.ones((128, 1), ml_dtypes.bfloat16)

    als = [float(np.asarray(inputs[f"al{i}"]).reshape(-1)[0]) for i in range(4)]
    bes = [float(np.asarray(inputs[f"be{i}"]).reshape(-1)[0]) for i in range(4)]
    return consts, include, als, bes


# ---------------------------------------------------------------- device code

CONST_SPECS = [
    ("dlhs", [128, NKB * MTOT], BF16),
    ("qwt1", [128, 3 * 365], BF16), ("qwt2", [128, 2 * 188], BF16),
    ("qwt3", [128, 99], BF16), ("qwt0", [128, 99], BF16),
    ("kwt1", [128, 3 * 365], BF16), ("kwt2", [128, 2 * 188], BF16),
    ("kwt3", [128, 99], BF16), ("kwt0", [128, 99], BF16),
    ("tmat1", [128, 3 * 365], BF16), ("tmat2", [128, 2 * 188], BF16),
    ("tmat3", [128, 99], BF16), ("tmat0", [128, 99], BF16),
    ("gt", [128, 7 * OUTW], BF16),
    ("outbias", [128, 3 * OUTW], F32),
    ("idx16", [BPC, 365], F32),
    ("ident16", [16, 16], F32),
    ("onehot16", [128, 16 * BPC], BF16),
    ("onesbf", [128, 1], BF16),
]


def build_nc(include, als, bes):
    nc = _build_nc_inner(include, als, bes)
    if SPLIT_WAITS:
        _split_excess_waits(nc)
    return nc


def _build_nc_inner(include, als, bes):
    nc = bass.Bass()
    x_p = nc.declare_dram_parameter("x", [BPC, SEQ, CH], BF16, isOutput=False)
    cparams = {}
    for nm, shp, dt in CONST_SPECS:
        cparams[nm] = nc.declare_dram_parameter(nm, shp, dt, isOutput=False)
    out_p = nc.declare_dram_parameter("out", [BPC, CH, OUTW], F32, isOutput=True)
    mean_p = nc.declare_dram_parameter("mean", [BPC, CH], F32, isOutput=True)
    std_p = nc.declare_dram_parameter("std", [BPC, CH], F32, isOutput=True)
    dbg = {}
    if DEBUG_OUTPUTS:
        for nm, shp in (("d_statrows", [BPC, 2 * CH]), ("d_rstd", [BPC, CH]),
                        ("d_comptu", [128, 7 * CH]), ("d_qs1", [128, 3 * 365]),
                        ("d_th1", [128, 3 * 365]), ("d_wt1", [128, 3 * BPC]),
                        ("d_cum1", [BPC, 365]), ("d_mask1", [BPC, 365]),
                        ("d_maskt", [128, 7 * BPC]), ("d_rstdt", [128, 3 * BPC])):
            dbg[nm] = nc.declare_dram_parameter(nm, shp, F32, isOutput=True)

    scales_order = [1, 2, 3, 0]
    LC = {i: (COMP_L[i] + 127) // 128 for i in range(4)}

    with tile.TileContext(nc) as tc:
        import contextlib
        with contextlib.ExitStack() as ctx:
            cpool = ctx.enter_context(tc.tile_pool(name="const", bufs=1))
            ppool = ctx.enter_context(tc.tile_pool(name="persist", bufs=1))

            # ---- constants into SBUF
            C = {}
            for nm, shp, dt in CONST_SPECS:
                t = cpool.tile(shp, dt, tag=nm, name=f"c_{nm}")
                nc.sync.dma_start(out=t[:], in_=cparams[nm][:])
                C[nm] = t

            # ---- persistent tiles
            compTu = [ppool.tile([128, 7 * CH], BF16, tag=f"compTu{b}",
                                 name=f"compTu{b}") for b in range(BPC)]
            WT16 = {i: ppool.tile([128, LC[i] * BPC], BF16, tag=f"wt16_{i}",
                                  name=f"wt16_{i}") for i in range(4)}
            statrows = ppool.tile([BPC, 2 * CH], F32, tag="statrows", name="statrows")
            meanstd = ppool.tile([BPC, 2 * CH], F32, tag="meanstd", name="meanstd")
            meanstdbf = ppool.tile([BPC, 2 * CH], BF16, tag="meanstdbf", name="meanstdbf")
            rstd16 = ppool.tile([BPC, CH], F32, tag="rstd16", name="rstd16")
            rstdT = ppool.tile([128, 3 * BPC], F32, tag="rstdT", name="rstdT")
            maskT = ppool.tile([128, 7 * BPC], F32, tag="maskT", name="maskT")
            mask16 = {i: ppool.tile([BPC, COMP_L[i]], F32, tag=f"m16_{i}",
                                    name=f"m16_{i}") for i in range(4)}

            xpool = ctx.enter_context(tc.tile_pool(name="xp", bufs=3))

            def load_x(b, with_meanstd):
                xt = xpool.tile([128, NKB * CH], BF16, tag="xt", name="xt")
                src = x_p[b, 0:640].rearrange("(k p) c -> p k c", p=128)
                dst = xt[0:128, 0:5 * CH].rearrange("p (k c) -> p k c", c=CH)
                nc.sync.dma_start(out=dst, in_=src)
                nc.sync.dma_start(out=xt[0:80, 5 * CH:6 * CH], in_=x_p[b, 640:720])
                if with_meanstd:
                    nc.sync.dma_start(out=xt[80:81, 5 * CH:6 * CH],
                                      in_=meanstdbf[b:b + 1, 0:CH])
                    nc.sync.dma_start(out=xt[81:82, 5 * CH:6 * CH],
                                      in_=meanstdbf[b:b + 1, CH:2 * CH])
                return xt

            # ================= Phase A: stats =================
            # Sums accumulate into PSUM partition b via a one-hot stationary
            # operand, so each sample's stats land in its own partition row.
            with tc.tile_pool(name="pa", bufs=2) as papool, \
                 tc.tile_pool(name="pap", bufs=1, space="PSUM") as paps, \
                 tc.tile_pool(name="papt", bufs=2, space="PSUM") as papt:
                ps_s = paps.tile([BPC, CH], F32, tag="pss", name="pss")
                ps_q = paps.tile([BPC, CH], F32, tag="psq", name="psq")
                for b in range(BPC):
                    xt = load_x(b, False)
                    xsq = papool.tile([128, NKB * CH], BF16, tag="xsq", name="xsq")
                    nc.vector.tensor_mul(xsq[:, 0:5 * CH], xt[:, 0:5 * CH],
                                         xt[:, 0:5 * CH])
                    nc.vector.tensor_mul(xsq[0:80, 5 * CH:6 * CH],
                                         xt[0:80, 5 * CH:6 * CH],
                                         xt[0:80, 5 * CH:6 * CH])
                    for k in range(NKB):
                        kp = 128 if k < 5 else 80
                        first = (b == 0 and k == 0)
                        last = (b == BPC - 1 and k == 5)
                        oh = C["onehot16"][0:kp, 16 * b:16 * b + 16]
                        nc.tensor.matmul(
                            ps_s[0:BPC, :], oh,
                            xt[0:kp, k * CH:(k + 1) * CH],
                            start=first, stop=last)
                        nc.tensor.matmul(
                            ps_q[0:BPC, :], oh,
                            xsq[0:kp, k * CH:(k + 1) * CH],
                            start=first, stop=last)
                nc.vector.tensor_copy(statrows[:, 0:CH], ps_s[0:BPC, :])
                nc.vector.tensor_copy(statrows[:, CH:2 * CH], ps_q[0:BPC, :])

                # batched: mean, var, rstd (Newton), std
                t1 = papool.tile([BPC, CH], F32, tag="t1", name="t1")
                t2 = papool.tile([BPC, CH], F32, tag="t2", name="t2")
                t3 = papool.tile([BPC, CH], F32, tag="t3", name="t3")
                nc.vector.tensor_scalar_mul(meanstd[:, 0:CH], statrows[:, 0:CH],
                                            1.0 / SEQ)
                nc.vector.tensor_scalar_mul(t1[:], statrows[:, CH:2 * CH], 1.0 / SEQ)
                nc.vector.tensor_mul(t2[:], meanstd[:, 0:CH], meanstd[:, 0:CH])
                # v = (msq + eps) - mean^2
                nc.vector.scalar_tensor_tensor(
                    t1[:], t1[:], 1e-5, t2[:],
                    op0=mybir.AluOpType.add, op1=mybir.AluOpType.subtract)
                # Newton rsqrt: y1 = 1.5 - 0.5 v ; 4 refinement steps
                y = rstd16
                nc.vector.tensor_scalar(y[:], t1[:], -0.5, 1.5,
                                        op0=mybir.AluOpType.mult,
                                        op1=mybir.AluOpType.add)
                for it in range(4):
                    nc.vector.tensor_mul(t2[:], y[:], y[:])
                    nc.vector.tensor_mul(t3[:], t2[:], t1[:])
                    nc.vector.tensor_scalar(t3[:], t3[:], -0.5, 1.5,
                                            op0=mybir.AluOpType.mult,
                                            op1=mybir.AluOpType.add)
                    nc.vector.tensor_mul(y[:], y[:], t3[:])
                nc.vector.tensor_mul(meanstd[:, CH:2 * CH], t1[:], y[:])  # std = v*y
                nc.vector.tensor_copy(meanstdbf[:], meanstd[:])
                if DEBUG_OUTPUTS:
                    nc.sync.dma_start(out=dbg["d_statrows"][:], in_=statrows[:])
                    nc.sync.dma_start(out=dbg["d_rstd"][:], in_=rstd16[:])
                nc.sync.dma_start(out=mean_p[:], in_=meanstd[:, 0:CH])
                nc.sync.dma_start(out=std_p[:], in_=meanstd[:, CH:2 * CH])
                # rstdT: [16, 321] -> [321, 16] in c-chunks
                for mc, (c0, cw) in enumerate(CCH):
                    pst = papt.tile([128, 16], F32, tag="ptr", name="ptr")
                    nc.tensor.transpose(pst[0:cw, 0:16],
                                        rstd16[0:BPC, c0:c0 + cw],
                                        C["ident16"][:])
                    nc.scalar.copy(rstdT[0:cw, mc * BPC:(mc + 1) * BPC],
                                   pst[0:cw, 0:16])
                if DEBUG_OUTPUTS:
                    nc.sync.dma_start(out=dbg["d_rstdt"][:], in_=rstdT[:])

            # ================= Phase B: per-sample heavy pipeline ============
            if STOP_AFTER == 'A':
                return nc
            with tc.tile_pool(name="pb", bufs=2) as pbpool, \
                 tc.tile_pool(name="pbp", bufs=2, space="PSUM") as pd, \
                 tc.tile_pool(name="pqkp", bufs=2, space="PSUM") as pqk, \
                 tc.tile_pool(name="psp", bufs=2, space="PSUM") as psc, \
                 tc.tile_pool(name="pwp", bufs=2, space="PSUM") as pwt:
                for b in range(BPC):
                    xt = load_x(b, True)
                    cu = compTu[b]
                    # ---- decomposition matmuls (float32r) + copies
                    for mb, (si, kb_l, row0, nr, aug) in enumerate(BLOCKS):
                        mw = MW[mb]
                        m0 = MOFF[mb]
                        ps = pd.tile([128, CH], F32, tag="pd", name="pd")
                        incl = include[mb]
                        for j, k in enumerate(incl):
                            kp = 128 if k < 5 else 82
                            nc.tensor.matmul(
                                ps[0:mw, :],
                                C["dlhs"][0:kp, k * MTOT + m0:k * MTOT + m0 + mw],
                                xt[0:kp, k * CH:(k + 1) * CH],
                                start=(j == 0), stop=(j == len(incl) - 1))
                        nc.vector.tensor_copy(cu[0:mw, mb * CH:(mb + 1) * CH],
                                              ps[0:mw, :])

                    # ---- Q/K with tanh (rstd as per-partition scale)
                    qk_sb = {}
                    for nm in ("q", "k"):
                        s1 = pbpool.tile([128, 3 * 365], BF16, tag=f"{nm}s1",
                                         name=f"{nm}s1")
                        sm = pbpool.tile([128, 3 * SMW], BF16, tag=f"{nm}sm",
                                         name=f"{nm}sm")
                        qk_sb[nm] = (s1, sm)
                        for mc, (c0, cw) in enumerate(CCH):
                            scale_ap = rstdT[0:cw, mc * BPC + b:mc * BPC + b + 1]
                            # scale 1 alone
                            pq = pqk.tile([128, SMW], F32, tag="pqk", name="pqk")
                            for jj, mb in enumerate(SCALE_BLOCKS[1]):
                                _, kb_l, row0, nr, aug = BLOCKS[mb]
                                kr = nr + (1 if aug else 0)
                                nc.tensor.matmul(
                                    pq[0:cw, 0:365],
                                    cu[0:kr, mb * CH + c0:mb * CH + c0 + cw],
                                    C[f"{nm}wt1"][0:kr, kb_l * 365:(kb_l + 1) * 365],
                                    start=(jj == 0), stop=(jj == 2))
                            nc.scalar.activation(
                                s1[0:cw, mc * 365:(mc + 1) * 365], pq[0:cw, 0:365],
                                mybir.ActivationFunctionType.Tanh, scale=scale_ap)
                            # small scales share a bank
                            pq2 = pqk.tile([128, SMW], F32, tag="pqk", name="pqk2")
                            for si in (2, 3, 0):
                                o = SM_OFF[si]
                                L = COMP_L[si]
                                mbs = SCALE_BLOCKS[si]
                                for jj, mb in enumerate(mbs):
                                    _, kb_l, row0, nr, aug = BLOCKS[mb]
                                    kr = nr + (1 if aug else 0)
                                    nc.tensor.matmul(
                                        pq2[0:cw, o:o + L],
                                        cu[0:kr, mb * CH + c0:mb * CH + c0 + cw],
                                        C[f"{nm}wt{si}"][0:kr, kb_l * L:(kb_l + 1) * L],
                                        start=(jj == 0), stop=(jj == len(mbs) - 1))
                            nc.scalar.activation(
                                sm[0:cw, mc * SMW:(mc + 1) * SMW], pq2[0:cw, 0:SMW],
                                mybir.ActivationFunctionType.Tanh, scale=scale_ap)

                    (qs1, qsm), (ks1, ksm) = qk_sb["q"], qk_sb["k"]

                    # ---- scores + tanh -> tanhS (bf16)
                    th1 = pbpool.tile([128, 3 * 365], BF16, tag="th1", name="th1")
                    thm = pbpool.tile([128, SMW + 188], BF16, tag="thm", name="thm")
                    for ml in range(3):               # scale 1, l-chunks
                        lw = [128, 128, 109][ml]
                        pss = psc.tile([128, SMW], F32, tag="ps", name="pss")
                        for mc, (c0, cw) in enumerate(CCH):
                            nc.tensor.matmul(
                                pss[0:lw, 0:365],
                                qs1[0:cw, mc * 365 + 128 * ml:
                                    mc * 365 + 128 * ml + lw],
                                ks1[0:cw, mc * 365:(mc + 1) * 365],
                                start=(mc == 0), stop=(mc == 2))
                        nc.scalar.activation(
                            th1[0:lw, ml * 365:ml * 365 + 365], pss[0:lw, 0:365],
                            mybir.ActivationFunctionType.Tanh,
                            scale=float(1.0 / np.sqrt(365.0)))
                    psa = psc.tile([128, SMW], F32, tag="ps", name="psa")
                    psb = psc.tile([128, SMW], F32, tag="ps", name="psb")
                    for si, lw, qoff, dsto, dstp in (
                            (2, 128, SM_OFF[2], 0, psa),
                            (3, 99, SM_OFF[3], 188, psa),
                            (0, 99, SM_OFF[0], 287, psa),
                            (2, 60, SM_OFF[2] + 128, 0, psb)):
                        L = COMP_L[si]
                        for mc, (c0, cw) in enumerate(CCH):
                            nc.tensor.matmul(
                                dstp[0:lw, dsto:dsto + L],
                                qsm[0:cw, mc * SMW + qoff:mc * SMW + qoff + lw],
                                ksm[0:cw, mc * SMW + SM_OFF[si]:
                                    mc * SMW + SM_OFF[si] + L],
                                start=(mc == 0), stop=(mc == 2))
                    nc.scalar.activation(
                        thm[0:128, 0:188], psa[0:128, 0:188],
                        mybir.ActivationFunctionType.Tanh,
                        scale=float(1.0 / np.sqrt(188.0)))
                    nc.scalar.activation(
                        thm[0:99, 188:386], psa[0:99, 188:386],
                        mybir.ActivationFunctionType.Tanh,
                        scale=float(1.0 / np.sqrt(99.0)))
                    nc.scalar.activation(
                        thm[0:60, SMW:SMW + 188], psb[0:60, 0:188],
                        mybir.ActivationFunctionType.Tanh,
                        scale=float(1.0 / np.sqrt(188.0)))

                    # ---- column means over l -> WT16 columns
                    def wt_mms(si, lhs_list):
                        """lhs_list: per l-chunk (ap, lw) covering [lw, L]"""
                        L = COMP_L[si]
                        nchunk = (L + 127) // 128
                        for ms in range(nchunk):
                            sw = min(128, L - 128 * ms)
                            pw = pwt.tile([128, 1], F32, tag="pw", name="pw")
                            for jj, (ap, lw) in enumerate(lhs_list):
                                nc.tensor.matmul(
                                    pw[0:sw, 0:1],
                                    ap[0:lw, 128 * ms:128 * ms + sw],
                                    C["onesbf"][0:lw, 0:1],
                                    start=(jj == 0), stop=(jj == len(lhs_list) - 1))
                            nc.vector.tensor_copy(
                                WT16[si][0:sw, ms * BPC + b:ms * BPC + b + 1],
                                pw[0:sw, 0:1])

                    wt_mms(1, [(th1[:, 0:365], 128), (th1[:, 365:730], 128),
                               (th1[:, 730:1095], 109)])
                    wt_mms(2, [(thm[:, 0:188], 128), (thm[:, SMW:SMW + 188], 60)])
                    wt_mms(3, [(thm[:, 188:287], 99)])
                    wt_mms(0, [(thm[:, 287:386], 99)])
                    if DEBUG_OUTPUTS and b == 0:
                        nc.gpsimd.dma_start(out=dbg["d_comptu"][:], in_=cu[:])
                        nc.gpsimd.dma_start(out=dbg["d_qs1"][:], in_=qs1[:])
                        nc.gpsimd.dma_start(out=dbg["d_th1"][:], in_=th1[:])

            # ================= Phase C: batched softmax / window =============
            if STOP_AFTER == 'B':
                return nc
            with tc.tile_pool(name="pc", bufs=2) as pcpool, \
                 tc.tile_pool(name="pcp", bufs=2, space="PSUM") as pcps:
                for si in scales_order:
                    L = COMP_L[si]
                    lc = LC[si]
                    pscm = pcps.tile([BPC, 512], F32, tag="pcum", name="pcum")
                    for kb in range(lc):
                        tw = min(128, L - 128 * kb)
                        lhsT = WT16[si][0:tw, kb * BPC:(kb + 1) * BPC]
                        rhs = C[f"tmat{si}"][0:tw, kb * L:(kb + 1) * L]
                        nc.tensor.matmul(pscm[0:BPC, 0:L], lhsT, rhs,
                                         start=(kb == 0), stop=(kb == lc - 1))
                    rmax = pcpool.tile([BPC, 1], F32, tag="rmax", name="rmax")
                    nc.vector.tensor_reduce(rmax[:], pscm[0:BPC, 0:L],
                                            axis=mybir.AxisListType.X,
                                            op=mybir.AluOpType.max)
                    nbias = pcpool.tile([BPC, 1], F32, tag="nbias", name="nbias")
                    nc.vector.tensor_scalar_mul(nbias[:], rmax[:], -als[si])
                    e16 = pcpool.tile([BPC, 365], F32, tag="e16", name="e16")
                    nc.scalar.activation(e16[0:BPC, 0:L], pscm[0:BPC, 0:L],
                                         mybir.ActivationFunctionType.Exp,
                                         bias=nbias[:], scale=als[si])
                    s16 = pcpool.tile([BPC, 1], F32, tag="s16", name="s16")
                    nc.vector.tensor_reduce(s16[:], e16[0:BPC, 0:L],
                                            axis=mybir.AxisListType.X,
                                            op=mybir.AluOpType.add)
                    prod = pcpool.tile([BPC, 365], F32, tag="prod", name="prod")
                    n16 = pcpool.tile([BPC, 1], F32, tag="n16", name="n16")
                    nc.vector.tensor_mul(prod[0:BPC, 0:L], e16[0:BPC, 0:L],
                                         C["idx16"][0:BPC, 0:L])
                    nc.vector.tensor_reduce(n16[:], prod[0:BPC, 0:L],
                                            axis=mybir.AxisListType.X,
                                            op=mybir.AluOpType.add)
                    rs = pcpool.tile([BPC, 1], F32, tag="rs", name="rs")
                    nc.vector.reciprocal(rs[:], s16[:])
                    win = pcpool.tile([BPC, 1], F32, tag="win", name="win")
                    nc.vector.tensor_mul(win[:], n16[:], rs[:])
                    tb = pcpool.tile([BPC, 1], F32, tag="tb", name="tb")
                    nc.vector.tensor_scalar_mul(tb[:], win[:], -bes[si] / 2.0)
                    nc.scalar.activation(mask16[si][0:BPC, 0:L],
                                         C["idx16"][0:BPC, 0:L],
                                         mybir.ActivationFunctionType.Tanh,
                                         bias=tb[:], scale=bes[si] / 2.0)
                    if DEBUG_OUTPUTS and si == 1:
                        nc.gpsimd.dma_start(out=dbg["d_wt1"][:], in_=WT16[1][:])
                        nc.vector.tensor_copy(prod[0:BPC, 0:L], pscm[0:BPC, 0:L])
                        nc.sync.dma_start(out=dbg["d_cum1"][0:BPC, 0:L], in_=prod[0:BPC, 0:L])
                        nc.sync.dma_start(out=dbg["d_mask1"][0:BPC, 0:L], in_=mask16[1][0:BPC, 0:L])
                # transposes to maskT (+ affine 0.5 x + 0.5)
                for mb, (si, kb_l, row0, nr, aug) in enumerate(BLOCKS):
                    pst = pcps.tile([128, 16], F32, tag="ptm", name="ptm")
                    nc.tensor.transpose(pst[0:nr, 0:16],
                                        mask16[si][0:BPC, 128 * kb_l:128 * kb_l + nr],
                                        C["ident16"][:])
                    nc.scalar.activation(maskT[0:nr, mb * BPC:(mb + 1) * BPC],
                                         pst[0:nr, 0:16],
                                         mybir.ActivationFunctionType.Copy,
                                         bias=0.5, scale=0.5)
                if DEBUG_OUTPUTS:
                    nc.sync.dma_start(out=dbg["d_maskt"][:], in_=maskT[:])

            # ================= Phase D: masked projection to output ==========
            with tc.tile_pool(name="pdl", bufs=3) as pdpool, \
                 tc.tile_pool(name="pdp", bufs=2, space="PSUM") as pdps:
                for b in range(BPC):
                    cu = compTu[b]
                    gm = pdpool.tile([128, 7 * OUTW], BF16, tag="gm", name="gm")
                    for mb, (si, kb_l, row0, nr, aug) in enumerate(BLOCKS):
                        nc.vector.tensor_scalar_mul(
                            gm[0:nr, mb * OUTW:(mb + 1) * OUTW],
                            C["gt"][0:nr, mb * OUTW:(mb + 1) * OUTW],
                            maskT[0:nr, mb * BPC + b:mb * BPC + b + 1])
                    po = pdps.tile([128, 3 * OUTW], F32, tag="po", name="po")
                    for mc, (c0, cw) in enumerate(CCH):
                        for mb in range(7):
                            nr = BLOCKS[mb][3]
                            nc.tensor.matmul(
                                po[0:cw, mc * OUTW:(mc + 1) * OUTW],
                                cu[0:nr, mb * CH + c0:mb * CH + c0 + cw],
                                gm[0:nr, mb * OUTW:(mb + 1) * OUTW],
                                start=(mb == 0), stop=(mb == 6))
                    osb = pdpool.tile([128, 3 * OUTW], F32, tag="osb", name="osb")
                    for mc, (c0, cw) in enumerate(CCH):
                        nc.vector.scalar_tensor_tensor(
                            osb[0:cw, mc * OUTW:(mc + 1) * OUTW],
                            po[0:cw, mc * OUTW:(mc + 1) * OUTW],
                            rstdT[0:cw, mc * BPC + b:mc * BPC + b + 1],
                            C["outbias"][0:cw, mc * OUTW:(mc + 1) * OUTW],
                            op0=mybir.AluOpType.mult, op1=mybir.AluOpType.add)
                    dst = out_p[b, 0:256].rearrange("(k p) j -> p k j", p=128)
                    src = osb[0:128, 0:2 * OUTW].rearrange("p (k j) -> p k j", j=OUTW)
                    nc.sync.dma_start(out=dst, in_=src)
                    nc.sync.dma_start(out=out_p[b, 256:CH],
                                      in_=osb[0:65, 2 * OUTW:3 * OUTW])
    return nc


# ---------------------------------------------------------------- entry point

def kernel(**inputs):
    x = np.ascontiguousarray(np.asarray(inputs["x"], np.float32))
    consts, include, als, bes = build_constants(inputs)
    nc = build_nc(include, als, bes)
    in_maps = []
    for c in range(N_CORES):
        m = dict(consts)
        m["x"] = np.ascontiguousarray(x[c * BPC:(c + 1) * BPC]).astype(
            ml_dtypes.bfloat16)
        in_maps.append(m)
    res = run_bass_kernel_spmd(nc, in_maps, list(range(N_CORES)))
    out = np.concatenate([res.results[c]["out"] for c in range(N_CORES)], axis=0)
    mean = np.concatenate([res.results[c]["mean"] for c in range(N_CORES)], axis=0)
    std = np.concatenate([res.results[c]["std"] for c in range(N_CORES)], axis=0)
    return (out.astype(np.float32),
            mean.reshape(BATCH, 1, CH).astype(np.float32),
            std.reshape(BATCH, 1, CH).astype(np.float32))
